# revision 42
# baseline (speedup 1.0000x reference)
"""Trainium2 Bass kernel for nn_CorrOptDiMPUnique (DiMP correlation-filter
steepest-descent optimizer, 2 iterations).

Sharding: data-parallel over the S=8 sequences, one per NeuronCore.

Per core the math is restructured around a Gram matrix:

  Phi[k=(j,c), q]  : 3x3x256 feature windows over the 25x25 padded grid
  scoresT = Phi^T fT                      [625, 529]   (fwd conv, iter 1 only)
  G       = Phi^T Phi                     [625, 625]   (computed once)
  gradT   = Phi RT + reg fT               [2304, 529]  (wgrad)
  sgT     = Phi^T gradT = G RT + reg scoresT            (scores_grad, cheap)
  scoresT' = scoresT - step * alpha * sgT               (iter-2 fwd conv free)

The residual uses the identity RT = mask * sw^2 * (s - label) with
mask = (sign(s + BIG*diag) + 1)/2 (act == s wherever mask != 0), removing
relu/copy_predicated from the critical chain. Everything lives in fp16
(alpha in bf16: values ~1e-4 would go subnormal in fp16), which halves
DMA bytes and enables the DVE 2x mode. The residual chain is processed in
two nf halves pipelined against PE matmuls; the fp16 filter update runs
eagerly per k-chunk on Vector inside the iter-2 wgrad loop (GpSimd is
avoided entirely: concurrent GpSimd tensor ops contend for SBUF ports and
slow Vector ~3x).
"""

import numpy as np

import concourse.bacc as bacc
import concourse.mybir as mybir
import concourse.tile as tile
from concourse.alu_op_type import AluOpType
from concourse.bass_utils import run_bass_kernel_spmd

F32 = mybir.dt.float32
F32R = mybir.dt.float32r
FP16 = mybir.dt.float16
BF16 = mybir.dt.bfloat16
AF = mybir.ActivationFunctionType

S, C, H, W, FSZ = 8, 256, 23, 23, 3
HW = H * W                      # 529
NF = HW
NFP = 530                       # nf padded to even
PW = W + 2                      # 25: padded grid width
QP = PW * PW                    # 625 padded positions
KK = C * FSZ * FSZ              # 2304
NKC = KK // 128                 # 18 k-chunks
MARG = 26                       # margin so shifted window reads stay in bounds
FPW = MARG + QP + MARG + 1      # 678 feature-pad width
NT = 5                          # q tiles
QT = 125                        # partitions per q tile
NW = NT * NFP                   # 2650: fused elementwise row width
NCH = [(0, 264), (264, 266)]    # nf halves (PSUM fp32 free-size <= 512)
GCH = [(0, 320), (320, 306)]    # q2 chunks for the Gram build
NUM_ITER = 2
MIN_FILTER_REG = 1e-5
NUM_BINS = 10
BIN_DISP = 0.5
DBIG = 30000.0                  # diag offset forcing sign(s + DBIG) = +1


def _host_maps(w_label: np.ndarray, w_spatial: np.ndarray):
    """[625, 530] label map / half spatial weight / diag index, numpy only."""
    dH, dW = 2 * H - 1, 2 * W - 1
    d0 = np.arange(dH, dtype=np.float32) - (dH // 2)
    d1 = np.arange(dW, dtype=np.float32) - (dW // 2)
    dist = np.sqrt(d0[:, None] ** 2 + d1[None, :] ** 2)
    bin_diff = dist[None] / BIN_DISP - np.arange(
        NUM_BINS, dtype=np.float32)[:, None, None]
    main = np.maximum(1.0 - np.abs(bin_diff[:-1]), 0.0)
    last = np.clip(1.0 + bin_diff[-1:], 0.0, 1.0)
    bins = np.concatenate([main, last], axis=0)
    label_full = np.einsum("b,bhw->hw", w_label.astype(np.float32), bins)
    sw_full = np.einsum("b,bhw->hw", w_spatial.astype(np.float32), bins)

    # m[(y,x),(i,j)] = full[H-1-i+y, W-1-j+x]  (symmetric in hw<->nf)
    yy = np.arange(H)
    iy = (H - 1) - yy[None, :] + yy[:, None]          # [y, i]
    ix = iy                                           # W == H
    lm = label_full[iy[:, None, :, None], ix[None, :, None, :]].reshape(HW, HW)
    sm = sw_full[iy[:, None, :, None], ix[None, :, None, :]].reshape(HW, HW)
    label_pad = np.zeros((QP, NFP), np.float16)
    swh_pad = np.zeros((QP, NFP), np.float16)
    hwq = np.full((QP, 1), -1.0, np.float32)
    yy2, xx2 = np.meshgrid(np.arange(H), np.arange(W), indexing="ij")
    qidx = ((yy2 + 1) * PW + (xx2 + 1)).ravel()       # padded index of real hw
    label_pad[qidx, :NF] = lm.astype(np.float16)
    swh_pad[qidx, :NF] = (0.5 * sm).astype(np.float16)
    hwq[qidx, 0] = np.arange(HW, dtype=np.float32)
    return label_pad, swh_pad, hwq, qidx


def _delta(j):  # flat padded-grid shift for kernel tap j = dy*3+dx
    dy, dx = j // 3, j % 3
    return (dy - 1) * PW + (dx - 1)


def _win(fpad_ap, t, j, width=QT, base=0):
    """[128, width] window into the padded feature (tap shift j, q tile t)."""
    o = MARG + _delta(j) + QT * t + base
    return fpad_ap[:, o:o + width]


def build_nc():
    nc = bacc.Bacc(None, target_bir_lowering=False)
    w_in = nc.dram_tensor("w_in", (KK, NFP), F32, kind="ExternalInput")
    # pre-padded feature rows (zeros baked in), fp32 for the f32r path
    f32p_in = nc.dram_tensor("f32p_in", (C, FPW), F32, kind="ExternalInput")
    phi_in = nc.dram_tensor("phi_in", (QP, KK), FP16, kind="ExternalInput")
    label_in = nc.dram_tensor("label_in", (QP, NFP), FP16, kind="ExternalInput")
    swh_in = nc.dram_tensor("swh_in", (QP, NFP), FP16, kind="ExternalInput")
    hwq_in = nc.dram_tensor("hwq_in", (QP, 1), F32, kind="ExternalInput")
    # scl_in[p,0] = -step_length, scl_in[p,1] = reg_weight (replicated rows)
    scl_in = nc.dram_tensor("scl_in", (128, 2), F32, kind="ExternalInput")
    f_out = nc.dram_tensor("f_out", (KK, NFP), FP16, kind="ExternalOutput")

    def TS(t, n0=0, nw=NFP):  # fused-tile slice for q-tile t, nf range
        return slice(t * NFP + n0, t * NFP + n0 + nw)

    with tile.TileContext(nc) as tc:
        with tc.tile_pool(name="big", bufs=1) as big:
            # ---------- persistent tiles ----------
            fpad = [big.tile([128, FPW], F32R, name=f"fpad{c2}")
                    for c2 in range(2)]
            phiT = [big.tile([QT, KK], FP16, name=f"phiT{t}") for t in range(NT)]
            G = [big.tile([QT, QP + 1], FP16, name=f"G{t}") for t in range(NT)]
            fTB32 = big.tile([128, NKC * NFP], F32R, name="fTB32")
            fTB = big.tile([128, NKC * NFP], FP16, name="fTB")
            gTB = big.tile([128, NKC * NFP], FP16, name="gTB")
            scB = big.tile([QT, NW], F32, name="scB")
            sgB = big.tile([QT, NW], F32, name="sgB")
            labB = big.tile([QT, NW], FP16, name="labB")
            swhB = big.tile([QT, NW], FP16, name="swhB")
            DbigB = big.tile([QT, NW], FP16, name="DbigB")
            sgnB = big.tile([QT, NW], FP16, name="sgnB")
            resB = big.tile([QT, NW], FP16, name="resB")
            swmB = big.tile([QT, NW], FP16, name="swmB")
            RTB = big.tile([QT, NW], FP16, name="RTB")
            sq2B = big.tile([QT, NW], BF16, name="sq2B")
            sqaccB = big.tile([128, NFP], F32, name="sqaccB")
            ones_col_f = big.tile([128, 1], F32, name="ones_col_f")
            alphaS = big.tile([128, NFP], BF16, name="alphaS")
            alphaS32 = big.tile([128, NFP], F32, name="alphaS32")
            hwq_sb = big.tile([128, NT], F32, name="hwq_sb")
            scl = big.tile([128, 2], F32, name="scl")
            nsb = big.tile([1, NFP], F32, name="nsb")
            den2 = big.tile([1, NFP], F32, name="den2")
            dn2r = big.tile([1, NFP], F32, name="dn2r")
            alpha_r = big.tile([1, NFP], F32R, name="alpha_r")
            ones_col_b = big.tile([128, 1], BF16, name="ones_col_b")
            ones_row_r = big.tile([1, 128], F32R, name="ones_row_r")
            ident = big.tile([128, 128], FP16, name="ident")

            def V(tb, ni):  # strided [QT, NT, w] view of a fused tile
                n0, w = NCH[ni]
                return tb.rearrange("q (t n) -> q t n", n=NFP)[:, :, n0:n0 + w]

            def abc_q(ni):  # alphaS32 broadcast over the t dim of a V view
                n0, w = NCH[ni]
                return alphaS32[:QT, n0:n0 + w].unsqueeze(1).broadcast_to(
                    (QT, NT, w))

            # ---------- input DMAs (ordered by when data is needed) ----------
            with tc.tile_pool(name="sup", bufs=1) as sup:
                iotac = sup.tile([128, NFP], F32, name="iotac")
                # padded fp32 feature first (gates the Gram build), rounded
                # to f32r
                fstage = [sup.tile([128, FPW], F32, name=f"fstage{c2}")
                          for c2 in range(2)]
                for c2 in range(2):
                    nc.sync.dma_start(
                        out=fstage[c2],
                        in_=f32p_in[c2 * 128:(c2 + 1) * 128, :])
                    nc.vector.tensor_copy(fpad[c2], fstage[c2])
                nc.sync.dma_start(out=scl, in_=scl_in[:, :])
                nc.sync.dma_start(
                    out=hwq_sb[:QT, :],
                    in_=hwq_in[:, :].rearrange("(t q) o -> q (t o)", t=NT))
                # filters: fp32 k-major staged in 3-kc chunks, rounded into
                # f32r fTB32 (verifier: f32r matmul inputs must be rounded)
                # plus an fp16 shadow for the update path
                w2 = 2 * NFP
                for h in range(9):
                    ws = sup.tile([128, w2], F32, tag="wst", bufs=3,
                                  name=f"wst{h}")
                    nc.sync.dma_start(
                        out=ws.rearrange("p (kc n) -> p kc n", n=NFP),
                        in_=w_in[:, :].rearrange(
                            "(kc p) n -> p kc n", kc=NKC)[:, h * 2:(h + 1) * 2])
                    nc.vector.tensor_copy(
                        fTB32[:, h * w2:(h + 1) * w2], ws)
                    nc.vector.tensor_copy(
                        fTB[:, h * w2:(h + 1) * w2], ws)
                nc.sync.dma_start(
                    out=labB.rearrange("q (t n) -> q t n", n=NFP),
                    in_=label_in[:, :].rearrange("(t q) n -> q t n", t=NT))
                nc.sync.dma_start(
                    out=swhB.rearrange("q (t n) -> q t n", n=NFP),
                    in_=swh_in[:, :].rearrange("(t q) n -> q t n", t=NT))
                for t in range(NT):
                    nc.sync.dma_start(
                        out=phiT[t], in_=phi_in[QT * t:QT * (t + 1), :])

                # diag mask (DBIG on the q==nf diagonal), identity, ones
                nc.gpsimd.iota(iotac, pattern=[[1, NFP]], base=0,
                               channel_multiplier=0,
                               allow_small_or_imprecise_dtypes=True)
                pidx = sup.tile([128, 1], F32, name="pidx")
                nc.gpsimd.iota(pidx, pattern=[[1, 1]], base=0,
                               channel_multiplier=1,
                               allow_small_or_imprecise_dtypes=True)
                for t in range(NT):
                    nc.vector.tensor_scalar(
                        out=DbigB[:, TS(t)], in0=iotac[:QT, :],
                        scalar1=hwq_sb[:QT, t:t + 1], scalar2=DBIG,
                        op0=AluOpType.is_equal, op1=AluOpType.mult)
                nc.vector.tensor_scalar(
                    out=ident, in0=iotac[:, :128], scalar1=pidx,
                    scalar2=None, op0=AluOpType.is_equal)
                ones_f = sup.tile([1, 128], F32, name="ones_f")
                nc.vector.memset(ones_f, 1.0)
                nc.vector.tensor_copy(ones_row_r, ones_f)
                nc.vector.memset(ones_col_f, 1.0)
                nc.vector.tensor_copy(ones_col_b, ones_col_f)

            # ---------- main ----------
            with (
                tc.tile_pool(name="wrk", bufs=1) as wrk,
                tc.tile_pool(name="psmm", bufs=2, space="PSUM") as psmm,
                tc.tile_pool(name="psred", bufs=1, space="PSUM") as psred,
            ):
                # --- Gram matrix G = Phi^T Phi: f32r matmuls over the upper
                # blocks only (even widths >= 256 per the f32r rule; t1 >= 3
                # rows widen their chunk instead of paying the narrow
                # penalty), stored fp16; lower blocks are PE-transposed
                # mirrors of the fp16 store.
                GROWCH = {0: GCH, 1: [(124, 502)], 2: [(250, 376)],
                          3: [(370, 256)], 4: [(370, 256)]}

                def build_G(t1):
                    chunks = GROWCH[t1]
                    pg = [psmm.tile([128, 512], F32, tag=f"a{gi % 2}",
                                    name=f"pg{t1}_{gi}")
                          for gi in range(len(chunks))]
                    for kc in range(NKC):
                        j, c2 = kc // 2, kc % 2
                        lhsT = _win(fpad[c2], t1, j)
                        for gi, (g0, gw) in enumerate(chunks):
                            nc.tensor.matmul(
                                pg[gi][:QT, :gw],
                                lhsT, _win(fpad[c2], 0, j, width=gw, base=g0),
                                start=(kc == 0), stop=(kc == NKC - 1))
                    for gi, (g0, gw) in enumerate(chunks):
                        nc.scalar.copy(
                            G[t1][:, g0:g0 + gw], pg[gi][:QT, :gw])

                def mirror_G(t1, t2):  # G[t2][:, t1-block] = G[t1] block^T
                    pt = psmm.tile([128, 512], F32, tag="a0",
                                   name=f"pt{t1}_{t2}")
                    ptv = pt.bitcast(FP16)
                    nc.tensor.transpose(
                        ptv[:QT, :QT],
                        G[t1][:, QT * t2:QT * t2 + QT], ident[:QT, :QT])
                    nc.scalar.copy(
                        G[t2][:, QT * t1:QT * t1 + QT], ptv[:QT, :QT])

                # --- forward conv iter 1 (one nf half): scoresT = Phi^T fT
                def fwd(ni):
                    n0, w = NCH[ni]
                    for t in range(NT):
                        ps = psmm.tile([128, 512], F32, tag=f"a{ni}",
                                       name=f"pf{t}_{ni}")
                        for kc in range(NKC):
                            j, c2 = kc // 2, kc % 2
                            nc.tensor.matmul(
                                ps[:QT, :w], _win(fpad[c2], t, j),
                                fTB32[:, kc * NFP + n0:kc * NFP + n0 + w],
                                start=(kc == 0), stop=(kc == NKC - 1))
                        nc.scalar.copy(scB[:, TS(t, n0, w)], ps[:QT, :w])

                # --- residual chain (one nf half, all fp16):
                #   RT = swm * sw*(s - label), swm = (sign(s + Dbig)+1)*swh
                def chain(it, ni):
                    if it == 1:
                        # scoresT_2 = scoresT + alphaS*sgT  (alphaS = -step*a)
                        nc.vector.tensor_mul(V(sgB, ni), V(sgB, ni), abc_q(ni))
                        nc.vector.tensor_add(V(scB, ni), V(scB, ni), V(sgB, ni))
                    nc.vector.tensor_add(V(sgnB, ni), V(scB, ni), V(DbigB, ni))
                    nc.scalar.activation(out=V(sgnB, ni), in_=V(sgnB, ni),
                                         func=AF.Sign)
                    nc.vector.tensor_sub(V(resB, ni), V(scB, ni), V(labB, ni))
                    nc.vector.scalar_tensor_tensor(
                        out=V(resB, ni), in0=V(resB, ni), scalar=2.0,
                        in1=V(swhB, ni), op0=AluOpType.mult, op1=AluOpType.mult)
                    nc.vector.scalar_tensor_tensor(
                        out=V(swmB, ni), in0=V(sgnB, ni), scalar=1.0,
                        in1=V(swhB, ni), op0=AluOpType.add, op1=AluOpType.mult)
                    nc.vector.tensor_mul(V(RTB, ni), V(swmB, ni), V(resB, ni))

                # --- sgT = G RT + reg*scoresT (one nf half) ---
                def sgt(it, ni):
                    n0, w = NCH[ni]
                    for t in range(NT):
                        ps = psmm.tile([128, 512], F32, tag=f"a{ni}",
                                       name=f"psg{it}_{t}_{ni}")
                        for tp in range(NT):
                            nc.tensor.matmul(
                                ps[:QT, :w], G[tp][:, QT * t:QT * t + QT],
                                RTB[:, TS(tp, n0, w)],
                                start=(tp == 0), stop=(tp == NT - 1))
                        nc.vector.scalar_tensor_tensor(
                            out=sgB[:, TS(t, n0, w)],
                            in0=scB[:, TS(t, n0, w)],
                            scalar=scl[:QT, 1:2], in1=ps[:QT, :w],
                            op0=AluOpType.mult, op1=AluOpType.add)

                # --- den += (swm*sgT)^2 (one nf half) ---
                def den_half(it, ni, den_ps):
                    n0, w = NCH[ni]
                    nc.vector.tensor_mul(V(resB, ni), V(swmB, ni), V(sgB, ni))
                    nc.scalar.activation(out=V(sq2B, ni), in_=V(resB, ni),
                                         func=AF.Square)
                    for t in range(NT):
                        nc.tensor.matmul(
                            den_ps[ni][:, :w], ones_col_b[:QT, :],
                            sq2B[:, TS(t, n0, w)],
                            start=(t == 0), stop=(t == NT - 1))

                # --- wgrad: gT = Phi RT + reg*fT ; num += gT^2 (squares
                # accumulated on Vector, reduced over k by one matmul pair).
                # it==1 folds the iter-1 filter update in per k-chunk:
                # fT2 = fT + alphaS*gT1 (eager, on Vector).
                def wgrad(it, num_ps):
                    for kc in range(NKC):
                        w0 = kc * NFP
                        if it == 1:
                            gv = gTB[:, w0:w0 + NFP]
                            nc.vector.tensor_mul(gv, gv, alphaS)
                            nc.vector.tensor_add(
                                fTB[:, w0:w0 + NFP], fTB[:, w0:w0 + NFP], gv)
                        ps = [psmm.tile([128, 512], F32, tag=f"a{ni}",
                                        name=f"pw{it}_{kc}_{ni}")
                              for ni in range(2)]
                        for t in range(NT):
                            lhsT = phiT[t][:, kc * 128:(kc + 1) * 128]
                            for ni, (n0, w) in enumerate(NCH):
                                nc.tensor.matmul(
                                    ps[ni][:, :w], lhsT,
                                    RTB[:, TS(t, n0, w)],
                                    start=(t == 0), stop=(t == NT - 1))
                        for ni, (n0, w) in enumerate(NCH):
                            nc.vector.scalar_tensor_tensor(
                                out=gTB[:, w0 + n0:w0 + n0 + w],
                                in0=fTB[:, w0 + n0:w0 + n0 + w],
                                scalar=scl[:, 1:2], in1=ps[ni][:, :w],
                                op0=AluOpType.mult, op1=AluOpType.add)
                        sq = wrk.tile([128, NFP], BF16, tag="sq", bufs=2)
                        nc.scalar.activation(
                            out=sq, in_=gTB[:, w0:w0 + NFP], func=AF.Square)
                        if kc == 0:
                            nc.vector.tensor_copy(sqaccB, sq)
                        else:
                            nc.vector.tensor_add(sqaccB, sqaccB, sq)
                    for ni, (n0, w) in enumerate(NCH):
                        nc.tensor.matmul(
                            num_ps[ni][:, :w], ones_col_f,
                            sqaccB[:, n0:n0 + w], start=True, stop=True)

                # --- alpha = -step * num / (den + reg*num), bcast to alphaS
                def alpha(it, num_ps, den_ps):
                    for ni, (n0, w) in enumerate(NCH):
                        nc.scalar.copy(nsb[:, n0:n0 + w], num_ps[ni][:, :w])
                        nc.vector.scalar_tensor_tensor(
                            out=den2[:, n0:n0 + w], in0=nsb[:, n0:n0 + w],
                            scalar=scl[0:1, 1:2], in1=den_ps[ni][:, :w],
                            op0=AluOpType.mult, op1=AluOpType.add)
                    nc.vector.tensor_scalar_max(den2, den2, 1e-8)
                    nc.vector.reciprocal_approx_fast(dn2r, den2)
                    nc.vector.scalar_tensor_tensor(
                        out=alpha_r, in0=nsb, scalar=scl[0:1, 0:1], in1=dn2r,
                        op0=AluOpType.mult, op1=AluOpType.mult)
                    for ni, (n0, w) in enumerate(NCH):
                        pb = psmm.tile([128, 512], F32, tag=f"a{ni}",
                                       name=f"pb{it}_{ni}")
                        nc.tensor.matmul(
                            pb[:, :w], ones_row_r, alpha_r[:, n0:n0 + w],
                            start=True, stop=True)
                        nc.scalar.copy(alphaS32[:, n0:n0 + w], pb[:, :w])
                        nc.scalar.copy(alphaS[:, n0:n0 + w], pb[:, :w])

                # ---------------- schedule ----------------
                den_ps_i = []
                num_ps_i = []
                for it in range(NUM_ITER):
                    den_ps_i.append([psred.tile([1, 512], F32, tag=f"den{ni}",
                                                name=f"den{it}_{ni}")
                                     for ni in range(2)])
                    num_ps_i.append([psred.tile([1, 512], F32, tag=f"num{ni}",
                                                name=f"num{it}_{ni}")
                                     for ni in range(2)])

                for t1 in (0, 1, 2, 3):  # PE work while w_in lands
                    build_G(t1)
                for t1, t2 in ((0, 1), (0, 2), (0, 3), (1, 2), (1, 3), (2, 3)):
                    mirror_G(t1, t2)
                fwd(0)
                build_G(4)               # PE cover for chain(0,0)
                for t1 in (0, 1, 2):     # (3,4) is covered by t1=4's chunk
                    mirror_G(t1, 4)
                chain(0, 0)
                fwd(1)                   # PE cover for chain(0,1)
                chain(0, 1)
                sgt(0, 0)
                den_half(0, 0, den_ps_i[0])
                sgt(0, 1)
                den_half(0, 1, den_ps_i[0])
                wgrad(0, num_ps_i[0])

                alpha(0, num_ps_i[0], den_ps_i[0])
                chain(1, 0)
                chain(1, 1)
                sgt(1, 0)
                den_half(1, 0, den_ps_i[1])
                sgt(1, 1)
                den_half(1, 1, den_ps_i[1])
                wgrad(1, num_ps_i[1])
                alpha(1, num_ps_i[1], den_ps_i[1])

                # ---- final: f = fT2 + alphaS2*gT2 -> DMA out (k-major) ----
                for h in range(6):
                    k0, k1 = h * 3, (h + 1) * 3
                    w0, w1 = k0 * NFP, k1 * NFP
                    gv = gTB[:, w0:w1].rearrange("p (k n) -> p k n", n=NFP)
                    nc.vector.tensor_mul(
                        gv, gv,
                        alphaS.unsqueeze(1).broadcast_to((128, 3, NFP)))
                    nc.vector.tensor_add(
                        fTB[:, w0:w1], fTB[:, w0:w1], gTB[:, w0:w1])
                    nc.sync.dma_start(
                        out=f_out[k0 * 128:k1 * 128, :].rearrange(
                            "(kc p) n -> p kc n", p=128),
                        in_=fTB[:, w0:w1].rearrange(
                            "p (kc n) -> p kc n", n=NFP))

    nc.compile()
    return nc


_NC_CACHE = {}


def _get_nc():
    if "nc" not in _NC_CACHE:
        _NC_CACHE["nc"] = build_nc()
    return _NC_CACHE["nc"]


def make_in_maps(filter, feat, w_label, w_spatial, log_step_length, filter_reg):
    filter = np.asarray(filter)
    feat = np.asarray(feat)
    label_m, swh_m, hwq, qidx = _host_maps(
        np.asarray(w_label), np.asarray(w_spatial))
    step = float(np.exp(np.asarray(log_step_length)))
    reg = float(max(float(np.asarray(filter_reg)) ** 2, MIN_FILTER_REG ** 2))
    scl = np.tile(np.array([[-step, reg]], np.float32), (128, 1))

    in_maps = []
    for s in range(S):
        # filter [NF, C, 3, 3] -> k-major [KK, NFP] with k = (dy*3+dx)*256 + c
        w_kp = np.zeros((KK, NFP), np.float32)
        w_kp[:, :NF] = filter[s].reshape(NF, C, 9).transpose(
            2, 1, 0).reshape(KK, NF)
        feat_s = feat[0, s].reshape(C, HW).astype(np.float32)
        # padded feature rows (fp32 + fp16) and phi windows
        # phi[q, j*C+c] = fpad[c, 26+q+delta(j)]
        f32p = np.zeros((C, FPW), np.float32)
        f32p[:, MARG + qidx] = feat_s
        f16p = f32p.astype(np.float16)
        phi = np.empty((QP, KK), np.float16)
        for j in range(9):
            o = MARG + _delta(j)
            phi[:, j * C:(j + 1) * C] = f16p[:, o:o + QP].T
        in_maps.append({
            "w_in": np.ascontiguousarray(w_kp),
            "f32p_in": f32p,
            "phi_in": phi,
            "label_in": label_m,
            "swh_in": swh_m,
            "hwq_in": hwq,
            "scl_in": scl,
        })
    return in_maps


def postprocess(results):
    outs = []
    for s in range(S):
        f_kp = results[s]["f_out"].astype(np.float32).reshape(
            9, C, NFP)[:, :, :NF]
        outs.append(np.ascontiguousarray(
            f_kp.transpose(2, 1, 0)).reshape(NF, C, FSZ, FSZ))
    return np.stack(outs, axis=0).astype(np.float32)


def kernel(filter, feat, w_label, w_spatial, log_step_length, filter_reg):
    in_maps = make_in_maps(filter, feat, w_label, w_spatial,
                           log_step_length, filter_reg)
    nc = _get_nc()
    res = run_bass_kernel_spmd(nc, in_maps, core_ids=list(range(S)))
    return postprocess(res.results)


# revision 44
# speedup vs baseline: 1.0630x; 1.0630x over previous
"""Trainium2 Bass kernel for nn_CorrOptDiMPUnique (DiMP correlation-filter
steepest-descent optimizer, 2 iterations).

Sharding: data-parallel over the S=8 sequences, one per NeuronCore.

Per core the math is restructured around a Gram matrix:

  Phi[k=(j,c), q]  : 3x3x256 feature windows over the 25x25 padded grid
  scoresT = Phi^T fT                      [625, 529]   (fwd conv, iter 1 only)
  G       = Phi^T Phi                     [625, 625]   (computed once)
  gradT   = Phi RT + reg fT               [2304, 529]  (wgrad)
  sgT     = Phi^T gradT = G RT + reg scoresT            (scores_grad, cheap)
  scoresT' = scoresT - step * alpha * sgT               (iter-2 fwd conv free)

The residual uses the identity RT = mask * sw^2 * (s - label) with
mask = (sign(s + BIG*diag) + 1)/2 (act == s wherever mask != 0), removing
relu/copy_predicated from the critical chain. Everything lives in fp16
(alpha in bf16: values ~1e-4 would go subnormal in fp16), which halves
DMA bytes and enables the DVE 2x mode. The residual chain is processed in
two nf halves pipelined against PE matmuls; the fp16 filter update runs
eagerly per k-chunk on Vector inside the iter-2 wgrad loop (GpSimd is
avoided entirely: concurrent GpSimd tensor ops contend for SBUF ports and
slow Vector ~3x).
"""

import numpy as np

import concourse.bacc as bacc
import concourse.mybir as mybir
import concourse.tile as tile
from concourse.alu_op_type import AluOpType
from concourse.bass_utils import run_bass_kernel_spmd

F32 = mybir.dt.float32
F32R = mybir.dt.float32r
FP16 = mybir.dt.float16
BF16 = mybir.dt.bfloat16
AF = mybir.ActivationFunctionType

S, C, H, W, FSZ = 8, 256, 23, 23, 3
HW = H * W                      # 529
NF = HW
NFP = 530                       # nf padded to even
PW = W + 2                      # 25: padded grid width
QP = PW * PW                    # 625 padded positions
KK = C * FSZ * FSZ              # 2304
NKC = KK // 128                 # 18 k-chunks
MARG = 26                       # margin so shifted window reads stay in bounds
FPW = MARG + QP + MARG + 1      # 678 feature-pad width
NT = 5                          # q tiles
QT = 125                        # partitions per q tile
NW = NT * NFP                   # 2650: fused elementwise row width
NCH = [(0, 264), (264, 266)]    # nf halves (PSUM fp32 free-size <= 512)
GCH = [(0, 320), (320, 306)]    # q2 chunks for the Gram build
NUM_ITER = 2
MIN_FILTER_REG = 1e-5
NUM_BINS = 10
BIN_DISP = 0.5
DBIG = 30000.0                  # diag offset forcing sign(s + DBIG) = +1


def _host_maps(w_label: np.ndarray, w_spatial: np.ndarray):
    """[625, 530] label map / half spatial weight / diag index, numpy only."""
    dH, dW = 2 * H - 1, 2 * W - 1
    d0 = np.arange(dH, dtype=np.float32) - (dH // 2)
    d1 = np.arange(dW, dtype=np.float32) - (dW // 2)
    dist = np.sqrt(d0[:, None] ** 2 + d1[None, :] ** 2)
    bin_diff = dist[None] / BIN_DISP - np.arange(
        NUM_BINS, dtype=np.float32)[:, None, None]
    main = np.maximum(1.0 - np.abs(bin_diff[:-1]), 0.0)
    last = np.clip(1.0 + bin_diff[-1:], 0.0, 1.0)
    bins = np.concatenate([main, last], axis=0)
    label_full = np.einsum("b,bhw->hw", w_label.astype(np.float32), bins)
    sw_full = np.einsum("b,bhw->hw", w_spatial.astype(np.float32), bins)

    # m[(y,x),(i,j)] = full[H-1-i+y, W-1-j+x]  (symmetric in hw<->nf)
    yy = np.arange(H)
    iy = (H - 1) - yy[None, :] + yy[:, None]          # [y, i]
    ix = iy                                           # W == H
    lm = label_full[iy[:, None, :, None], ix[None, :, None, :]].reshape(HW, HW)
    sm = sw_full[iy[:, None, :, None], ix[None, :, None, :]].reshape(HW, HW)
    label_pad = np.zeros((QP, NFP), np.float16)
    swh_pad = np.zeros((QP, NFP), np.float16)
    hwq = np.full((QP, 1), -1.0, np.float32)
    yy2, xx2 = np.meshgrid(np.arange(H), np.arange(W), indexing="ij")
    qidx = ((yy2 + 1) * PW + (xx2 + 1)).ravel()       # padded index of real hw
    label_pad[qidx, :NF] = lm.astype(np.float16)
    swh_pad[qidx, :NF] = (0.5 * sm).astype(np.float16)
    hwq[qidx, 0] = np.arange(HW, dtype=np.float32)
    return label_pad, swh_pad, hwq, qidx


def _delta(j):  # flat padded-grid shift for kernel tap j = dy*3+dx
    dy, dx = j // 3, j % 3
    return (dy - 1) * PW + (dx - 1)


def _win(fpad_ap, t, j, width=QT, base=0):
    """[128, width] window into the padded feature (tap shift j, q tile t)."""
    o = MARG + _delta(j) + QT * t + base
    return fpad_ap[:, o:o + width]


def build_nc():
    nc = bacc.Bacc(None, target_bir_lowering=False)
    w_in = nc.dram_tensor("w_in", (KK, NFP), F32, kind="ExternalInput")
    # pre-padded feature rows (zeros baked in), fp32 for the f32r path
    f32p_in = nc.dram_tensor("f32p_in", (C, FPW), F32, kind="ExternalInput")
    phi_in = nc.dram_tensor("phi_in", (QP, KK), FP16, kind="ExternalInput")
    label_in = nc.dram_tensor("label_in", (QP, NFP), FP16, kind="ExternalInput")
    swh_in = nc.dram_tensor("swh_in", (QP, NFP), FP16, kind="ExternalInput")
    hwq_in = nc.dram_tensor("hwq_in", (QP, 1), F32, kind="ExternalInput")
    # scl_in[p,0] = -step_length, scl_in[p,1] = reg_weight (replicated rows)
    scl_in = nc.dram_tensor("scl_in", (128, 2), F32, kind="ExternalInput")
    f_out = nc.dram_tensor("f_out", (KK, NFP), FP16, kind="ExternalOutput")

    def TS(t, n0=0, nw=NFP):  # fused-tile slice for q-tile t, nf range
        return slice(t * NFP + n0, t * NFP + n0 + nw)

    with tile.TileContext(nc) as tc:
        with tc.tile_pool(name="big", bufs=1) as big:
            # ---------- persistent tiles ----------
            fpad = [big.tile([128, FPW], F32R, name=f"fpad{c2}")
                    for c2 in range(2)]
            phiT = [big.tile([QT, KK], FP16, name=f"phiT{t}") for t in range(NT)]
            G = [big.tile([QT, QP + 1], FP16, name=f"G{t}") for t in range(NT)]
            fTB32 = big.tile([128, NKC * NFP], F32R, name="fTB32")
            fTB = big.tile([128, NKC * NFP], FP16, name="fTB")
            gTB = big.tile([128, NKC * NFP], FP16, name="gTB")
            scB = big.tile([QT, NW], F32, name="scB")
            sgB = big.tile([QT, NW], F32, name="sgB")
            labB = big.tile([QT, NW], FP16, name="labB")
            swhB = big.tile([QT, NW], FP16, name="swhB")
            DbigB = big.tile([QT, NW], FP16, name="DbigB")
            sgnB = big.tile([QT, NW], FP16, name="sgnB")
            resB = big.tile([QT, NW], FP16, name="resB")
            swmB = big.tile([QT, NW], FP16, name="swmB")
            RTB = big.tile([QT, NW], FP16, name="RTB")
            sq2B = big.tile([QT, NW], BF16, name="sq2B")
            ones_col_f = big.tile([128, 1], F32, name="ones_col_f")
            alphaS = big.tile([128, NFP], BF16, name="alphaS")
            alphaS32 = big.tile([128, NFP], F32, name="alphaS32")
            hwq_sb = big.tile([128, NT], F32, name="hwq_sb")
            scl = big.tile([128, 2], F32, name="scl")
            nsb = big.tile([1, NFP], F32, name="nsb")
            den2 = big.tile([1, NFP], F32, name="den2")
            dn2r = big.tile([1, NFP], F32, name="dn2r")
            alpha_r = big.tile([1, NFP], F32R, name="alpha_r")
            ones_col_b = big.tile([128, 1], BF16, name="ones_col_b")
            ones_row_r = big.tile([1, 128], F32R, name="ones_row_r")
            ident = big.tile([128, 128], FP16, name="ident")

            def V(tb, ni):  # strided [QT, NT, w] view of a fused tile
                n0, w = NCH[ni]
                return tb.rearrange("q (t n) -> q t n", n=NFP)[:, :, n0:n0 + w]

            def abc_q(ni):  # alphaS32 broadcast over the t dim of a V view
                n0, w = NCH[ni]
                return alphaS32[:QT, n0:n0 + w].unsqueeze(1).broadcast_to(
                    (QT, NT, w))

            # ---------- input DMAs (ordered by when data is needed) ----------
            with tc.tile_pool(name="sup", bufs=1) as sup:
                iotac = sup.tile([128, NFP], F32, name="iotac")
                # padded fp32 feature first (gates the Gram build), rounded
                # to f32r
                fstage = [sup.tile([128, FPW], F32, name=f"fstage{c2}")
                          for c2 in range(2)]
                for c2 in range(2):
                    nc.sync.dma_start(
                        out=fstage[c2],
                        in_=f32p_in[c2 * 128:(c2 + 1) * 128, :])
                    nc.vector.tensor_copy(fpad[c2], fstage[c2])
                nc.sync.dma_start(out=scl, in_=scl_in[:, :])
                nc.sync.dma_start(
                    out=hwq_sb[:QT, :],
                    in_=hwq_in[:, :].rearrange("(t q) o -> q (t o)", t=NT))
                # filters: fp32 k-major staged in 3-kc chunks, rounded into
                # f32r fTB32 (verifier: f32r matmul inputs must be rounded)
                # plus an fp16 shadow for the update path
                w2 = 2 * NFP
                for h in range(9):
                    ws = sup.tile([128, w2], F32, tag="wst", bufs=3,
                                  name=f"wst{h}")
                    nc.sync.dma_start(
                        out=ws.rearrange("p (kc n) -> p kc n", n=NFP),
                        in_=w_in[:, :].rearrange(
                            "(kc p) n -> p kc n", kc=NKC)[:, h * 2:(h + 1) * 2])
                    nc.vector.tensor_copy(
                        fTB32[:, h * w2:(h + 1) * w2], ws)
                    nc.vector.tensor_copy(
                        fTB[:, h * w2:(h + 1) * w2], ws)
                nc.sync.dma_start(
                    out=labB.rearrange("q (t n) -> q t n", n=NFP),
                    in_=label_in[:, :].rearrange("(t q) n -> q t n", t=NT))
                nc.sync.dma_start(
                    out=swhB.rearrange("q (t n) -> q t n", n=NFP),
                    in_=swh_in[:, :].rearrange("(t q) n -> q t n", t=NT))
                for t in range(NT):
                    nc.sync.dma_start(
                        out=phiT[t], in_=phi_in[QT * t:QT * (t + 1), :])

                # diag mask (DBIG on the q==nf diagonal), identity, ones
                nc.gpsimd.iota(iotac, pattern=[[1, NFP]], base=0,
                               channel_multiplier=0,
                               allow_small_or_imprecise_dtypes=True)
                pidx = sup.tile([128, 1], F32, name="pidx")
                nc.gpsimd.iota(pidx, pattern=[[1, 1]], base=0,
                               channel_multiplier=1,
                               allow_small_or_imprecise_dtypes=True)
                for t in range(NT):
                    nc.vector.tensor_scalar(
                        out=DbigB[:, TS(t)], in0=iotac[:QT, :],
                        scalar1=hwq_sb[:QT, t:t + 1], scalar2=DBIG,
                        op0=AluOpType.is_equal, op1=AluOpType.mult)
                nc.vector.tensor_scalar(
                    out=ident, in0=iotac[:, :128], scalar1=pidx,
                    scalar2=None, op0=AluOpType.is_equal)
                ones_f = sup.tile([1, 128], F32, name="ones_f")
                nc.vector.memset(ones_f, 1.0)
                nc.vector.tensor_copy(ones_row_r, ones_f)
                nc.vector.memset(ones_col_f, 1.0)
                nc.vector.tensor_copy(ones_col_b, ones_col_f)

            # ---------- main ----------
            with (
                tc.tile_pool(name="wrk", bufs=1) as wrk,
                tc.tile_pool(name="psmm", bufs=2, space="PSUM") as psmm,
                tc.tile_pool(name="psred", bufs=1, space="PSUM") as psred,
            ):
                # --- Gram matrix G = Phi^T Phi: f32r matmuls over the upper
                # blocks only (even widths >= 256 per the f32r rule; t1 >= 3
                # rows widen their chunk instead of paying the narrow
                # penalty), stored fp16; lower blocks are PE-transposed
                # mirrors of the fp16 store.
                GROWCH = {0: GCH, 1: [(124, 502)], 2: [(250, 376)],
                          3: [(370, 256)], 4: [(370, 256)]}

                def build_G(t1):
                    chunks = GROWCH[t1]
                    pg = [psmm.tile([128, 512], F32, tag=f"a{gi % 2}",
                                    name=f"pg{t1}_{gi}")
                          for gi in range(len(chunks))]
                    for kc in range(NKC):
                        j, c2 = kc // 2, kc % 2
                        lhsT = _win(fpad[c2], t1, j)
                        for gi, (g0, gw) in enumerate(chunks):
                            nc.tensor.matmul(
                                pg[gi][:QT, :gw],
                                lhsT, _win(fpad[c2], 0, j, width=gw, base=g0),
                                start=(kc == 0), stop=(kc == NKC - 1))
                    for gi, (g0, gw) in enumerate(chunks):
                        nc.scalar.copy(
                            G[t1][:, g0:g0 + gw], pg[gi][:QT, :gw])

                def mirror_G(t1, t2):  # G[t2][:, t1-block] = G[t1] block^T
                    pt = psmm.tile([128, 512], F32, tag="a0",
                                   name=f"pt{t1}_{t2}")
                    ptv = pt.bitcast(FP16)
                    nc.tensor.transpose(
                        ptv[:QT, :QT],
                        G[t1][:, QT * t2:QT * t2 + QT], ident[:QT, :QT])
                    nc.scalar.copy(
                        G[t2][:, QT * t1:QT * t1 + QT], ptv[:QT, :QT])

                # --- forward conv iter 1 (one nf half): scoresT = Phi^T fT
                def fwd(ni):
                    n0, w = NCH[ni]
                    for t in range(NT):
                        ps = psmm.tile([128, 512], F32, tag=f"a{ni}",
                                       name=f"pf{t}_{ni}")
                        for kc in range(NKC):
                            j, c2 = kc // 2, kc % 2
                            nc.tensor.matmul(
                                ps[:QT, :w], _win(fpad[c2], t, j),
                                fTB32[:, kc * NFP + n0:kc * NFP + n0 + w],
                                start=(kc == 0), stop=(kc == NKC - 1))
                        nc.scalar.copy(scB[:, TS(t, n0, w)], ps[:QT, :w])

                # --- residual chain (one nf half, all fp16):
                #   RT = swm * sw*(s - label), swm = (sign(s + Dbig)+1)*swh
                def chain(it, ni):
                    if it == 1:
                        # scoresT_2 = scoresT + alphaS*sgT  (alphaS = -step*a)
                        nc.vector.tensor_mul(V(sgB, ni), V(sgB, ni), abc_q(ni))
                        nc.vector.tensor_add(V(scB, ni), V(scB, ni), V(sgB, ni))
                    nc.vector.tensor_add(V(sgnB, ni), V(scB, ni), V(DbigB, ni))
                    nc.scalar.activation(out=V(sgnB, ni), in_=V(sgnB, ni),
                                         func=AF.Sign)
                    nc.vector.tensor_sub(V(resB, ni), V(scB, ni), V(labB, ni))
                    nc.vector.scalar_tensor_tensor(
                        out=V(resB, ni), in0=V(resB, ni), scalar=2.0,
                        in1=V(swhB, ni), op0=AluOpType.mult, op1=AluOpType.mult)
                    nc.vector.scalar_tensor_tensor(
                        out=V(swmB, ni), in0=V(sgnB, ni), scalar=1.0,
                        in1=V(swhB, ni), op0=AluOpType.add, op1=AluOpType.mult)
                    nc.vector.tensor_mul(V(RTB, ni), V(swmB, ni), V(resB, ni))

                # --- sgT = G RT + reg*scoresT (one nf half) ---
                def sgt(it, ni):
                    n0, w = NCH[ni]
                    for t in range(NT):
                        ps = psmm.tile([128, 512], F32, tag=f"a{ni}",
                                       name=f"psg{it}_{t}_{ni}")
                        for tp in range(NT):
                            nc.tensor.matmul(
                                ps[:QT, :w], G[tp][:, QT * t:QT * t + QT],
                                RTB[:, TS(tp, n0, w)],
                                start=(tp == 0), stop=(tp == NT - 1))
                        nc.vector.scalar_tensor_tensor(
                            out=sgB[:, TS(t, n0, w)],
                            in0=scB[:, TS(t, n0, w)],
                            scalar=scl[:QT, 1:2], in1=ps[:QT, :w],
                            op0=AluOpType.mult, op1=AluOpType.add)

                # --- den += (swm*sgT)^2 (one nf half) ---
                def den_half(it, ni, den_ps):
                    n0, w = NCH[ni]
                    nc.vector.tensor_mul(V(resB, ni), V(swmB, ni), V(sgB, ni))
                    nc.scalar.activation(out=V(sq2B, ni), in_=V(resB, ni),
                                         func=AF.Square)
                    for t in range(NT):
                        nc.tensor.matmul(
                            den_ps[ni][:, :w], ones_col_b[:QT, :],
                            sq2B[:, TS(t, n0, w)],
                            start=(t == 0), stop=(t == NT - 1))

                # --- wgrad: gT = Phi RT + reg*fT ; num += gT^2 (squares
                # accumulated on Vector, reduced over k by one matmul pair).
                # it==1 folds the iter-1 filter update in per k-chunk:
                # fT2 = fT + alphaS*gT1 (eager, on Vector).
                def wgrad(it, num_ps):
                    for kc in range(NKC):
                        w0 = kc * NFP
                        if it == 1:
                            gv = gTB[:, w0:w0 + NFP]
                            nc.vector.tensor_mul(gv, gv, alphaS)
                            nc.vector.tensor_add(
                                fTB[:, w0:w0 + NFP], fTB[:, w0:w0 + NFP], gv)
                        ps = [psmm.tile([128, 512], F32, tag=f"a{ni}",
                                        name=f"pw{it}_{kc}_{ni}")
                              for ni in range(2)]
                        for t in range(NT):
                            lhsT = phiT[t][:, kc * 128:(kc + 1) * 128]
                            for ni, (n0, w) in enumerate(NCH):
                                nc.tensor.matmul(
                                    ps[ni][:, :w], lhsT,
                                    RTB[:, TS(t, n0, w)],
                                    start=(t == 0), stop=(t == NT - 1))
                        for ni, (n0, w) in enumerate(NCH):
                            nc.vector.scalar_tensor_tensor(
                                out=gTB[:, w0 + n0:w0 + n0 + w],
                                in0=fTB[:, w0 + n0:w0 + n0 + w],
                                scalar=scl[:, 1:2], in1=ps[ni][:, :w],
                                op0=AluOpType.mult, op1=AluOpType.add)
                        sq = wrk.tile([128, NFP], BF16, tag="sq", bufs=2)
                        nc.scalar.activation(
                            out=sq, in_=gTB[:, w0:w0 + NFP], func=AF.Square)
                        for ni, (n0, w) in enumerate(NCH):
                            nc.tensor.matmul(
                                num_ps[ni][:, :w], ones_col_b,
                                sq[:, n0:n0 + w],
                                start=(kc == 0), stop=(kc == NKC - 1))

                # --- alpha = -step * num / (den + reg*num), bcast to alphaS
                def alpha(it, num_ps, den_ps):
                    for ni, (n0, w) in enumerate(NCH):
                        nc.scalar.copy(nsb[:, n0:n0 + w], num_ps[ni][:, :w])
                        nc.vector.scalar_tensor_tensor(
                            out=den2[:, n0:n0 + w], in0=nsb[:, n0:n0 + w],
                            scalar=scl[0:1, 1:2], in1=den_ps[ni][:, :w],
                            op0=AluOpType.mult, op1=AluOpType.add)
                    nc.vector.tensor_scalar_max(den2, den2, 1e-8)
                    nc.vector.reciprocal_approx_fast(dn2r, den2)
                    nc.vector.scalar_tensor_tensor(
                        out=alpha_r, in0=nsb, scalar=scl[0:1, 0:1], in1=dn2r,
                        op0=AluOpType.mult, op1=AluOpType.mult)
                    for ni, (n0, w) in enumerate(NCH):
                        pb = psmm.tile([128, 512], F32, tag=f"a{ni}",
                                       name=f"pb{it}_{ni}")
                        nc.tensor.matmul(
                            pb[:, :w], ones_row_r, alpha_r[:, n0:n0 + w],
                            start=True, stop=True)
                        nc.scalar.copy(alphaS32[:, n0:n0 + w], pb[:, :w])
                        nc.scalar.copy(alphaS[:, n0:n0 + w], pb[:, :w])

                # ---------------- schedule ----------------
                den_ps_i = []
                num_ps_i = []
                for it in range(NUM_ITER):
                    den_ps_i.append([psred.tile([1, 512], F32, tag=f"den{ni}",
                                                name=f"den{it}_{ni}")
                                     for ni in range(2)])
                    num_ps_i.append([psred.tile([1, 512], F32, tag=f"num{ni}",
                                                name=f"num{it}_{ni}")
                                     for ni in range(2)])

                for t1 in (0, 1, 2, 3):  # PE work while w_in lands
                    build_G(t1)
                for t1, t2 in ((0, 1), (0, 2), (0, 3), (1, 2), (1, 3), (2, 3)):
                    mirror_G(t1, t2)
                fwd(0)
                build_G(4)               # PE cover for chain(0,0)
                for t1 in (0, 1, 2):     # (3,4) is covered by t1=4's chunk
                    mirror_G(t1, 4)
                chain(0, 0)
                fwd(1)                   # PE cover for chain(0,1)
                chain(0, 1)
                sgt(0, 0)
                den_half(0, 0, den_ps_i[0])
                sgt(0, 1)
                den_half(0, 1, den_ps_i[0])
                wgrad(0, num_ps_i[0])

                alpha(0, num_ps_i[0], den_ps_i[0])
                chain(1, 0)
                chain(1, 1)
                sgt(1, 0)
                den_half(1, 0, den_ps_i[1])
                sgt(1, 1)
                den_half(1, 1, den_ps_i[1])
                wgrad(1, num_ps_i[1])
                alpha(1, num_ps_i[1], den_ps_i[1])

                # ---- final: f = fT2 + alphaS2*gT2 -> DMA out (k-major) ----
                for h in range(6):
                    k0, k1 = h * 3, (h + 1) * 3
                    w0, w1 = k0 * NFP, k1 * NFP
                    gv = gTB[:, w0:w1].rearrange("p (k n) -> p k n", n=NFP)
                    nc.vector.tensor_mul(
                        gv, gv,
                        alphaS.unsqueeze(1).broadcast_to((128, 3, NFP)))
                    nc.vector.tensor_add(
                        fTB[:, w0:w1], fTB[:, w0:w1], gTB[:, w0:w1])
                    nc.sync.dma_start(
                        out=f_out[k0 * 128:k1 * 128, :].rearrange(
                            "(kc p) n -> p kc n", p=128),
                        in_=fTB[:, w0:w1].rearrange(
                            "p (kc n) -> p kc n", n=NFP))

    nc.compile()
    return nc


_NC_CACHE = {}


def _get_nc():
    if "nc" not in _NC_CACHE:
        _NC_CACHE["nc"] = build_nc()
    return _NC_CACHE["nc"]


def make_in_maps(filter, feat, w_label, w_spatial, log_step_length, filter_reg):
    filter = np.asarray(filter)
    feat = np.asarray(feat)
    label_m, swh_m, hwq, qidx = _host_maps(
        np.asarray(w_label), np.asarray(w_spatial))
    step = float(np.exp(np.asarray(log_step_length)))
    reg = float(max(float(np.asarray(filter_reg)) ** 2, MIN_FILTER_REG ** 2))
    scl = np.tile(np.array([[-step, reg]], np.float32), (128, 1))

    in_maps = []
    for s in range(S):
        # filter [NF, C, 3, 3] -> k-major [KK, NFP] with k = (dy*3+dx)*256 + c
        w_kp = np.zeros((KK, NFP), np.float32)
        w_kp[:, :NF] = filter[s].reshape(NF, C, 9).transpose(
            2, 1, 0).reshape(KK, NF)
        feat_s = feat[0, s].reshape(C, HW).astype(np.float32)
        # padded feature rows (fp32 + fp16) and phi windows
        # phi[q, j*C+c] = fpad[c, 26+q+delta(j)]
        f32p = np.zeros((C, FPW), np.float32)
        f32p[:, MARG + qidx] = feat_s
        f16p = f32p.astype(np.float16)
        phi = np.empty((QP, KK), np.float16)
        for j in range(9):
            o = MARG + _delta(j)
            phi[:, j * C:(j + 1) * C] = f16p[:, o:o + QP].T
        in_maps.append({
            "w_in": np.ascontiguousarray(w_kp),
            "f32p_in": f32p,
            "phi_in": phi,
            "label_in": label_m,
            "swh_in": swh_m,
            "hwq_in": hwq,
            "scl_in": scl,
        })
    return in_maps


def postprocess(results):
    outs = []
    for s in range(S):
        f_kp = results[s]["f_out"].astype(np.float32).reshape(
            9, C, NFP)[:, :, :NF]
        outs.append(np.ascontiguousarray(
            f_kp.transpose(2, 1, 0)).reshape(NF, C, FSZ, FSZ))
    return np.stack(outs, axis=0).astype(np.float32)


def kernel(filter, feat, w_label, w_spatial, log_step_length, filter_reg):
    in_maps = make_in_maps(filter, feat, w_label, w_spatial,
                           log_step_length, filter_reg)
    nc = _get_nc()
    res = run_bass_kernel_spmd(nc, in_maps, core_ids=list(range(S)))
    return postprocess(res.results)
